# revision 2
# baseline (speedup 1.0000x reference)
"""Trainium2 Bass kernel for nn_Atten_RNN: fp16 weights/activations on the
matmul paths (fp32 PSUM accumulation + fp32 softmax/bias), leaner attention.

Sharding: batch-parallel (B=64 -> 8 per core) for RNN + attention; vocab-
parallel (32000 -> 4000 per core) for W_out, joined by one feat AllGather.

RNN is weight-stationary in the h^T layout: per step 64 fp16 LDW+matmul pairs
(K=128, M=128, N=8) accumulate pre^T in one PSUM bank, then one DVE add
(x-projection, fp32) and one tanh -> hT (fp16) directly.
"""

import numpy as np
from contextlib import ExitStack

import concourse.bass as bass
import concourse.tile as tile
from concourse import bacc, mybir
from concourse.bass_utils import run_bass_kernel_spmd
from concourse.masks import make_identity

FP = mybir.dt.float32
F16 = mybir.dt.float16
I16 = mybir.dt.int16

N_CORES = 8
B = 64
B_LOC = B // N_CORES          # 8
S_FULL = 512
E = 512
H = 1024
V = 32000
V_SH = V // N_CORES           # 4000
KC = H // 128                 # 8 hidden chunks
QC = E // 128                 # 4 embedding chunks
Tanh = mybir.ActivationFunctionType.Tanh
Exp = mybir.ActivationFunctionType.Exp


def build_nc(S=S_FULL, n_cores=N_CORES, collective=True):
    NT = S * B_LOC            # tokens per core, t = s*B_LOC + b
    ST = S // 128             # attention s-tiles
    FW = 128 * KC * B_LOC     # 8192: flattened hT row width
    assert S % 128 == 0 and NT % 128 == 0

    nc = bacc.Bacc("TRN2", target_bir_lowering=False, debug=False,
                   num_devices=n_cores)

    # ---- external I/O (per core) ----
    x_idx = nc.dram_tensor("x_idx", [128, NT // 16], I16, kind="ExternalInput")
    emb_t = nc.dram_tensor("emb_t", [V, E], F16, kind="ExternalInput")
    w_ihT = nc.dram_tensor("w_ihT", [QC, 128, H], F16, kind="ExternalInput")
    w_hhT = nc.dram_tensor("w_hhT", [KC, 128, H], F16, kind="ExternalInput")
    bias_pb = nc.dram_tensor("bias_pb", [128, KC], FP, kind="ExternalInput")
    w_outT = nc.dram_tensor("w_outT", [16, 128, V_SH], F16, kind="ExternalInput")
    b_out_sh = nc.dram_tensor("b_out_sh", [1, V_SH], F16, kind="ExternalInput")
    y_out = nc.dram_tensor("y_out", [B, V_SH], FP, kind="ExternalOutput")

    # ---- internal DRAM ----
    xw_dram = nc.dram_tensor("xw_dram", [KC, NT // 512, 128, 512], F16)
    out2 = nc.dram_tensor("out2", [S, FW], F16)    # row s = hT(s) flat (p, kc, b)
    ag_in = nc.dram_tensor("ag_in", [B_LOC, 2 * H], FP)
    ag_out = nc.dram_tensor("ag_out", [B, 2 * H], FP, addr_space="Shared")

    with tile.TileContext(nc) as tc, ExitStack() as top:
        consts = top.enter_context(tc.tile_pool(name="consts", bufs=1))
        ident = consts.tile([128, 128], F16)
        make_identity(nc, ident[:])
        ones_row = consts.tile([1, 128], F16)
        nc.vector.memset(ones_row[:], 1.0)
        bias_pb_sb = consts.tile([128, KC], FP)
        nc.sync.dma_start(bias_pb_sb[:], bias_pb[:, :])
        bout_sb = consts.tile([1, V_SH], F16)
        nc.sync.dma_start(bout_sb[:], b_out_sh[0:1, :])
        lastT_sb = consts.tile([128, KC * B_LOC], F16)

        # ================= phase 1+2: gather + transpose -> xeT =================
        with tc.tile_pool(name="xeT_p", bufs=1) as xeT_p, \
             tc.tile_pool(name="wih_p", bufs=1) as wih_p:
            wih_sb = wih_p.tile([128, QC, H], F16)
            nc.sync.dma_start(wih_sb[:], w_ihT.ap().rearrange("q p h -> p q h"))
            xeT_sb = xeT_p.tile([128, QC, NT], F16)
            with tc.tile_pool(name="xe_p", bufs=1) as xe_p, \
                 tc.tile_pool(name="idx_p", bufs=1) as idx_p, \
                 tc.tile_pool(name="trp_p", bufs=4, space="PSUM") as trp_p:
                xidx_sb = idx_p.tile([128, NT // 16], I16)
                nc.sync.dma_start(xidx_sb[:], x_idx[:, :])
                xe_sb = xe_p.tile([128, NT // 128, E], F16)
                GCH = 1024  # indices per dma_gather (keeps SWDGE ring within capacity)
                for g in range(NT // GCH):
                    nc.gpsimd.dma_gather(
                        out_ap=xe_sb[:, g * (GCH // 128):(g + 1) * (GCH // 128), :],
                        in_ap=emb_t.ap(),
                        idxs_ap=xidx_sb[:, g * (GCH // 16):(g + 1) * (GCH // 16)],
                        num_idxs=GCH, num_idxs_reg=GCH, elem_size=E)
                for c in range(NT // 128):
                    for q in range(QC):
                        pt = trp_p.tile([128, 128], F16)
                        nc.tensor.transpose(pt[:], xe_sb[:, c, q * 128:(q + 1) * 128], ident[:])
                        nc.vector.tensor_copy(xeT_sb[:, q, c * 128:(c + 1) * 128], pt[:])

            # ========= phase 3: xW^T[h, t] = W_ih @ xe^T + bias (hT layout) =========
            with tc.tile_pool(name="xw_ps", bufs=2, space="PSUM") as xw_ps, \
                 tc.tile_pool(name="xw_ev", bufs=3) as xw_ev:
                for hc in range(KC):
                    for tcn in range(NT // 512):
                        ps = xw_ps.tile([128, 512], FP)
                        for q in range(QC):
                            nc.tensor.matmul(
                                ps[:], wih_sb[:, q, hc * 128:(hc + 1) * 128],
                                xeT_sb[:, q, tcn * 512:(tcn + 1) * 512],
                                start=(q == 0), stop=(q == QC - 1))
                        ev = xw_ev.tile([128, 512], F16)
                        nc.vector.tensor_scalar_add(ev[:], ps[:], bias_pb_sb[:, hc:hc + 1])
                        nc.sync.dma_start(xw_dram[hc, tcn, :, :], ev[:])

        # ================= phase 4: RNN (weight-stationary, hT layout) =========
        # Split each step into two half-psum groups (mc 0..3 | mc 4..7) so
        # tanh of half A hides behind the matmuls of half B, and batch hT
        # stores 8 steps per DMA from a staging tile.
        HB = KC // 2 * B_LOC          # 32 cols per half
        with tc.tile_pool(name="whh_p", bufs=1) as whh_p, \
             tc.tile_pool(name="stage_p", bufs=3) as stage_p, \
             tc.tile_pool(name="xwb_p", bufs=2) as xwb_p, \
             tc.tile_pool(name="rnn_ps", bufs=4, space="PSUM") as rnn_ps:
            whh_sb = whh_p.tile([128, KC, H], F16)
            nc.sync.dma_start(whh_sb[:], w_hhT.ap().rearrange("k p h -> p k h"))
            h_init = stage_p.tile([128, 8, KC * B_LOC], F16, tag="stage")
            nc.vector.memset(h_init[:, 7, :], 0.0)
            prevA = h_init[:, 7, 0:HB]
            prevB = h_init[:, 7, HB:2 * HB]
            xwb = None
            stage = None
            for s in range(S):
                if s % 64 == 0:
                    blk = s // 64
                    xwb = xwb_p.tile([128, KC, 512], F16, tag="xwb")
                    nc.sync.dma_start(xwb[:], xw_dram[:, blk, :, :].rearrange(
                        "k p t -> p k t"))
                j = s % 8
                if j == 0:
                    stage = stage_p.tile([128, 8, KC * B_LOC], F16, tag="stage")
                xw_sl = xwb[:].rearrange("p k (si b) -> p si k b", b=B_LOC)
                halves = []
                for hf in range(2):
                    ph = rnn_ps.tile([128, HB], FP, tag="rnn_psum")
                    nc.tensor.matmul(
                        ph[:], ident[:], xw_sl[:, s % 64, hf * 4:(hf + 1) * 4, :],
                        start=True, stop=False, skip_group_check=True)
                    for kc in range(KC):
                        hp_src = prevA if kc < 4 else prevB
                        for mc in range(hf * 4, (hf + 1) * 4):
                            nc.tensor.matmul(
                                ph[:, (mc - hf * 4) * B_LOC:(mc - hf * 4 + 1) * B_LOC],
                                whh_sb[:, kc, mc * 128:(mc + 1) * 128],
                                hp_src[:, (kc % 4) * B_LOC:(kc % 4 + 1) * B_LOC],
                                start=False, stop=(kc == KC - 1 and mc == (hf + 1) * 4 - 1),
                                skip_group_check=True)
                    nc.scalar.activation(
                        stage[:, j, hf * HB:(hf + 1) * HB], ph[:], Tanh)
                    halves.append(ph)
                prevA = stage[:, j, 0:HB]
                prevB = stage[:, j, HB:2 * HB]
                if j == 7:
                    nc.gpsimd.dma_start(
                        out2[s - 7:s + 1, :].rearrange("a (p f) -> p a f", p=128),
                        stage[:])
            nc.vector.tensor_copy(lastT_sb[:], stage[:, 7, :])

        # ================= phase 5: attention =================
        with tc.tile_pool(name="att_sb", bufs=1) as att_sb:
          with tc.tile_pool(name="o2_p", bufs=1) as o2_p, \
               tc.tile_pool(name="att_ps", bufs=2, space="PSUM") as att_ps:
            # resident out2 in SBUF: [128, ST, FW] f16 (64KB/partition)
            o2all = o2_p.tile([128, ST, FW], F16)
            nc.sync.dma_start(
                o2all[:], out2.ap().rearrange("(c p) f -> p c f", p=128))

            # broadcast lastT across partitions: lastB[p, FW] f16
            lastrow_sb = att_sb.tile([1, FW], F16)
            nc.sync.dma_start(
                lastrow_sb[:], out2[S - 1:S, :])
            lastB = att_sb.tile([128, FW], F16)
            for i in range(FW // 512):
                lb = att_ps.tile([128, 512], FP, tag="attps")
                nc.tensor.matmul(lb[:], ones_row[:, 0:128],
                                 lastrow_sb[0:1, i * 512:(i + 1) * 512],
                                 start=True, stop=True)
                nc.vector.tensor_copy(lastB[:, i * 512:(i + 1) * 512], lb[:])

            # scores[s-chunk, b] = sum_h hT_s * last  (DVE mul+reduce, f16 in)
            scoresS = att_sb.tile([128, ST * B_LOC], FP)
            with tc.tile_pool(name="prod_p", bufs=2) as prod_p:
                for c in range(ST):
                    pr = prod_p.tile([128, FW], F16, tag="prod")
                    nc.vector.tensor_mul(pr[:], o2all[:, c, :], lastB[:])
                    nc.vector.reduce_sum(
                        scoresS[:, c * B_LOC:(c + 1) * B_LOC],
                        pr[:].rearrange("s (hp kc b) -> s b hp kc", hp=128, kc=KC, b=B_LOC),
                        axis=mybir.AxisListType.XY)

            # softmax over time (b on partitions)
            identf = att_sb.tile([128, 128], FP)
            make_identity(nc, identf[:])
            scoresT = att_sb.tile([B_LOC, S], FP)
            for c in range(ST):
                sp = att_ps.tile([B_LOC, 128], FP, tag="attps")
                nc.tensor.transpose(sp[:], scoresS[:, c * B_LOC:(c + 1) * B_LOC], identf[:])
                nc.vector.tensor_copy(scoresT[:, c * 128:(c + 1) * 128], sp[:])
            nc.vector.memset(scoresT[:, S - 1:S], -1e30)  # step S-1 excluded
            negmax = att_sb.tile([B_LOC, 1], FP)
            nc.vector.reduce_max(negmax[:], scoresT[:], axis=mybir.AxisListType.X, negate=True)
            expT = att_sb.tile([B_LOC, S], FP)
            nc.scalar.activation(expT[:], scoresT[:], Exp, bias=negmax[:])
            ssum = att_sb.tile([B_LOC, 1], FP)
            nc.vector.reduce_sum(ssum[:], expT[:], axis=mybir.AxisListType.X)
            rinv = att_sb.tile([B_LOC, 1], FP)
            nc.vector.reciprocal(rinv[:], ssum[:])
            attnT = att_sb.tile([B_LOC, S], F16)
            nc.vector.tensor_scalar_mul(attnT[:], expT[:], rinv[:])
            attnS = att_sb.tile([128, ST, B_LOC], F16)
            for c in range(ST):
                ap_ = att_ps.tile([128, B_LOC], F16, tag="attps16")
                nc.tensor.transpose(ap_[:], attnT[:, c * 128:(c + 1) * 128],
                                    ident[0:B_LOC, 0:B_LOC])
                nc.vector.tensor_copy(attnS[:, c, :], ap_[:])

            # att rows: feat[b, 0:H] = sum_s attn[s,b] * h_s[b, :]
            # per (b, half): 4 accumulating matmuls K=128 (s-chunk), N=512.
            # DVE cannot write partition-offset slices, so each [1,512] psum
            # result is DMA'd straight into its ag_in row.
            o2r = o2all[:].rearrange("s c (hp kc b) -> s c hp kc b",
                                     hp=128, kc=KC, b=B_LOC)
            with tc.tile_pool(name="attm_ps", bufs=4, space="PSUM") as attm_ps, \
                 tc.tile_pool(name="attrow_p", bufs=2) as attrow_p:
                for b in range(B_LOC):
                    rb = attrow_p.tile([1, H], FP, tag="attrow")
                    for half in range(2):
                        pa = attm_ps.tile([1, 512], FP, tag="attm")
                        for c in range(ST):
                            nc.tensor.matmul(
                                pa[:], attnS[:, c, b:b + 1],
                                o2r[:, c, :, half * 4:(half + 1) * 4, b]
                                  .rearrange("s hp kc -> s kc hp"),
                                start=(c == 0), stop=(c == ST - 1))
                        nc.vector.tensor_copy(
                            rb[0:1, half * 512:(half + 1) * 512], pa[:])
                    nc.sync.dma_start(ag_in[b:b + 1, 0:H], rb[:])

            # last rows: feat[b, H:2H] = h_last[b, :]
            feat_last = att_sb.tile([B_LOC, H], FP)
            for i in range(KC):
                fp_ = att_ps.tile([B_LOC, 128], F16, tag="attps16")
                nc.tensor.transpose(
                    fp_[:], lastT_sb[:, i * B_LOC:(i + 1) * B_LOC], ident[:])
                nc.vector.tensor_copy(feat_last[:, i * 128:(i + 1) * 128], fp_[:])

            nc.sync.dma_start(ag_in[:, H:2 * H], feat_last[:])
            if collective:
                nc.gpsimd.collective_compute(
                    "AllGather", mybir.AluOpType.bypass,
                    replica_groups=[list(range(n_cores))],
                    ins=[ag_in.ap()], outs=[ag_out.ap()])
            else:  # timeline-sim variant: fake the gather with local copies
                for cc in range(n_cores):
                    nc.sync.dma_start(ag_out[cc * B_LOC:(cc + 1) * B_LOC, :], ag_in[:, :])
            featfull = att_sb.tile([B, 2 * H], FP)
            nc.sync.dma_start(featfull[:], ag_out[:, :])
            featT_full = att_sb.tile([128, 16, B], F16)
            for i in range(16):
                fq = att_ps.tile([128, B], FP, tag="attps")
                nc.tensor.transpose(fq[:], featfull[:, i * 128:(i + 1) * 128],
                                    identf[0:B, 0:B])
                nc.vector.tensor_copy(featT_full[:, i, :], fq[:])

          # ================= phase 6: projection =================
          NV = V_SH // 8  # 500-wide psum chunks
          with tc.tile_pool(name="wo_p", bufs=4) as wo_p, \
               tc.tile_pool(name="y_ps", bufs=1, space="PSUM") as y_ps, \
               tc.tile_pool(name="y_sb_p", bufs=1) as y_sb_p:
              psums = [y_ps.tile([B, NV], FP, tag=f"y{n}", name=f"ypsum{n}")
                       for n in range(8)]
              for kc in range(16):
                  wot = wo_p.tile([128, V_SH], F16, tag="wot")
                  nc.sync.dma_start(wot[:], w_outT[kc, :, :])
                  for n in range(8):
                      nc.tensor.matmul(psums[n][:], featT_full[:, kc, :],
                                       wot[:, n * NV:(n + 1) * NV],
                                       start=(kc == 0), stop=False)
              for n in range(8):
                  nc.tensor.matmul(psums[n][:], ones_row[:, 0:B],
                                   bout_sb[0:1, n * NV:(n + 1) * NV],
                                   start=False, stop=True)
              y_sb = y_sb_p.tile([B, V_SH], FP)
              for n in range(8):
                  nc.vector.tensor_copy(y_sb[:, n * NV:(n + 1) * NV], psums[n][:])
              nc.sync.dma_start(y_out[:, :], y_sb[:])

    nc.compile()
    return nc


def host_prep(X, emb, W_ih, W_hh, b_ih, b_hh, W_out, b_out, S=S_FULL, n_cores=N_CORES):
    """Build the per-core input maps (sharding + fp16 layout prep on host)."""
    NT = S * B_LOC
    emb_f = np.ascontiguousarray(np.asarray(emb, np.float32).astype(np.float16))
    w_ihT = np.ascontiguousarray(
        np.asarray(W_ih, np.float32).T.astype(np.float16).reshape(QC, 128, H))
    w_hhT = np.ascontiguousarray(
        np.asarray(W_hh, np.float32).T.astype(np.float16).reshape(KC, 128, H))
    bias_pb = np.ascontiguousarray(
        (np.asarray(b_ih, np.float32) + np.asarray(b_hh, np.float32)).reshape(KC, 128).T)
    in_maps = []
    for c in range(n_cores):
        Xl = np.asarray(X[c * B_LOC:(c + 1) * B_LOC, :S])
        tok = Xl.T.reshape(-1)                        # t = s*B_LOC + b
        idx = np.zeros((128, NT // 16), np.int16)
        for g in range(8):
            idx[g * 16:(g + 1) * 16, :] = tok.reshape(NT // 16, 16).T
        Wo = np.asarray(W_out[c * V_SH:(c + 1) * V_SH, :], np.float32)
        w_outT = np.ascontiguousarray(Wo.T.astype(np.float16).reshape(16, 128, V_SH))
        in_maps.append({
            "x_idx": idx,
            "emb_t": emb_f,
            "w_ihT": w_ihT,
            "w_hhT": w_hhT,
            "bias_pb": bias_pb,
            "w_outT": w_outT,
            "b_out_sh": np.asarray(b_out[c * V_SH:(c + 1) * V_SH],
                                   np.float32).astype(np.float16).reshape(1, V_SH),
        })
    return in_maps


_NC_CACHE = {}


def kernel(X, emb, W_ih, W_hh, b_ih, b_hh, W_out, b_out):
    X = np.asarray(X)
    in_maps = host_prep(X, emb, W_ih, W_hh, b_ih, b_hh, W_out, b_out)
    if "nc" not in _NC_CACHE:
        _NC_CACHE["nc"] = build_nc()
    nc = _NC_CACHE["nc"]
    res = run_bass_kernel_spmd(nc, in_maps, list(range(N_CORES)))
    Y = np.concatenate([res.results[i]["y_out"] for i in range(N_CORES)], axis=1)
    return Y.astype(np.float32)


if __name__ == "__main__":
    import importlib.util
    spec = importlib.util.spec_from_file_location("reference", "/root/problem/reference.py")
    ref = importlib.util.module_from_spec(spec)
    spec.loader.exec_module(ref)
    inputs = {k: np.asarray(v) for k, v in ref.setup_inputs().items()}
    Y = kernel(**inputs)
    print(Y.shape, Y.dtype)
